# revision 29
# baseline (speedup 1.0000x reference)
"""Trainium2 Bass kernel for windowed multi-head attention with relative
position bias (Swin-style block):

    qkv = x @ qkv_w.T + [q_bias, 0, v_bias]
    q, k, v = split(qkv);  q *= hd**-0.5
    attn = softmax(q @ k.T + rel_table[rel_index])
    out  = (attn @ v) @ proj_w.T + proj_b

Shapes: x [8, 32, 32, 768], 12 heads, head_dim 64, N=1024 tokens.

Sharding: pure data-parallel — one batch element per NeuronCore, 8 cores,
no collectives. Each core runs an identical NEFF on its own slice.

v2 design (single software-pipelined stream, everything fp16 on the PE):
  All inputs host-cast to fp16; all matmuls run at 1 col/cycle with the
  PE as the single critical engine (~175us of PE work).  Emission order
  interleaves the QKV projections, per-head attention, and normalization
  so ACT (exp), DVE (bias mults/evictions) and Pool (SBUF mults) all hang
  off the PE stream without ever blocking it:

    V(all 8 t-tiles) ; Q(0) K(0)
    cycle j in 0..5 (heads a=2j, b=2j+1):
      Q(j+1) | S(a)+exp+mult | K(j+1) | AV(a) | S(b)+exp+mult |
      evict(a) | AV(b) | evict(b) | pairnorm(j)
    tail: proj(0..5) + bias evict + yT out

  S uses K=64 contraction directly from the natural k_t layout (no
  zero-padded K copies, no big memsets).  The softmax bias is folded
  multiplicatively (host ships exp(bias)^T fp16); normalization is
  deferred: AV's ones-column yields row sums, each pair's inverse is
  computed on ACT via exp(-ln(s)), bounced through a tiny DRAM scratch to
  partition-broadcast it, and applied with one fp16 multiply per pair.
  PSUM: 2x[128,1024] S sets + 2x[128,1024] general matmul bufs = 8 banks.
"""

import numpy as np

_CACHE = {}

B = 8
WS = 32
N = WS * WS            # 1024 tokens
C = 768
NH = 12
HD = 64
P = 128
QC = 2                 # q chunks of 512
QN = N // QC           # 512
KT = N // P            # 8 k tiles
CT = C // P            # 6 contraction tiles
NJ = NH // 2           # 6 head pairs


def _build():
    import concourse.bass as bass
    import concourse.bacc as bacc
    import concourse.mybir as mybir
    import concourse.tile as tile
    from concourse.bass import _add_dep_helper

    f32 = mybir.dt.float32
    f16 = mybir.dt.float16
    AF = mybir.ActivationFunctionType

    nc = bacc.Bacc(None, target_bir_lowering=False)

    xT_d = nc.dram_tensor("xT", [C, N], f16, kind="ExternalInput")
    wqk_d = nc.dram_tensor("wqk", [C, 2 * C], f16, kind="ExternalInput")
    wv_d = nc.dram_tensor("wv", [C, C], f16, kind="ExternalInput")
    wproj_d = nc.dram_tensor("wproj", [C, C], f16, kind="ExternalInput")
    qkb_d = nc.dram_tensor("qkb", [CT, P], f32, kind="ExternalInput")
    vb_d = nc.dram_tensor("vb", [C], f32, kind="ExternalInput")
    pb_d = nc.dram_tensor("pb", [CT, P], f32, kind="ExternalInput")
    biasT_d = nc.dram_tensor("biasT", [NH, N, N], f16, kind="ExternalInput")
    yT_d = nc.dram_tensor("yT", [C, N], f16, kind="ExternalOutput")
    inv_d = nc.dram_tensor("inv_scr", [NJ, 2, N], f16)

    with tile.TileContext(nc) as tc:
        with (
            tc.tile_pool(name="cst", bufs=1) as cst,
            tc.tile_pool(name="bias_pool", bufs=2) as bias_pool,
            tc.tile_pool(name="pt_pool", bufs=2) as pt_pool,
            tc.tile_pool(name="sums_pool", bufs=2) as sums_pool,
            tc.tile_pool(name="inv_pool", bufs=2) as inv_pool,
            tc.tile_pool(name="pinv_pool", bufs=2) as pinv_pool,
            tc.tile_pool(name="yb_pool", bufs=2) as yb_pool,
            tc.tile_pool(name="ps_s", bufs=3, space="PSUM") as ps_s,
            tc.tile_pool(name="ps_mm", bufs=1, space="PSUM") as ps_mm,
        ):
            # ---- permanent SBUF ----
            xT = cst.tile([P, CT, N], f16)
            wqk = cst.tile([P, CT, 2 * C], f16)
            wv = cst.tile([P, CT, C], f16)
            wproj = cst.tile([P, CT, C], f16)
            q_t = cst.tile([P, CT, N], f16)        # Q^T rows = channels
            k_t = cst.tile([P, CT, N], f16)        # K^T rows = channels
            v_aug = cst.tile([P, KT, NH, HD + 1], f16)  # V + ones column
            attn_outT = cst.tile([P, CT, N], f16)  # unnormalized AV output
            qkb = cst.tile([P, CT], f32)
            vb_bc = cst.tile([P, C], f32)
            pbias = cst.tile([P, CT], f32)

            nc.gpsimd.memset(v_aug[:, :, :, HD:HD + 1], 1.0)

            # ---- input DMAs: x/wv first (V phase gate) on sync; small
            # constants + bias table on the idle gpsimd queue ----
            xT_src = xT_d[:].rearrange("(k p) t -> p k t", p=P)
            wv_src = wv_d[:].rearrange("(k p) o -> p k o", p=P)
            wqk_src = wqk_d[:].rearrange("(k p) o -> p k o", p=P)
            wproj_src = wproj_d[:].rearrange("(k p) o -> p k o", p=P)
            # Round-robin xT/wv across the SP, ACT and DVE queues (all idle at
            # start) so the config chains run in parallel and V's k-slices
            # land in consumption order as early as possible.
            qs = [nc.sync, nc.scalar, nc.gpsimd]
            for k in range(CT):
                qs[(2 * k) % 3].dma_start(xT[:, k, :], xT_src[:, k, :])
                qs[(2 * k + 1) % 3].dma_start(wv[:, k, :], wv_src[:, k, :])
            nc.gpsimd.dma_start(qkb, qkb_d[:].rearrange("j p -> p j"))
            nc.gpsimd.dma_start(
                vb_bc, bass.AP(tensor=vb_d, offset=0, ap=[[0, P], [1, C]]))
            nc.gpsimd.dma_start(pbias, pb_d[:].rearrange("j p -> p j"))

            biasT = {}

            def load_bias(h):
                biasT[h] = bias_pool.tile([P, KT, N], f16, tag="biasT",
                                          name=f"biasT{h}")
                nc.gpsimd.dma_start(
                    biasT[h], biasT_d[h].rearrange("(kt p) q -> p kt q", p=P))



            # ---- V phase: V natural [t, o], fp16, via the mm pool ----
            for tt in range(KT):
                pool = ps_s if tt % 2 == 0 else ps_mm
                mmv = pool.tile([P, N], f32,
                                tag="pss" if tt % 2 == 0 else "mm",
                                name=f"mmv{tt}")
                for k in range(CT):
                    for vc in range(2):
                        nc.tensor.matmul(
                            mmv[:, vc * QN:vc * QN + 384],
                            xT[:, k, tt * P:(tt + 1) * P],
                            wv[:, k, vc * 384:(vc + 1) * 384],
                            start=(k == 0), stop=(k == CT - 1))
                for vc in range(2):
                    nc.vector.tensor_add(
                        v_aug[:, tt, vc * 6:(vc + 1) * 6, 0:HD],
                        mmv[:, vc * QN:vc * QN + 384].rearrange(
                            "p (h d) -> p h d", d=HD),
                        vb_bc[:, vc * 384:(vc + 1) * 384].rearrange(
                            "p (h d) -> p h d", d=HD))

            for k in range(CT):
                nc.scalar.dma_start(wqk[:, k, :], wqk_src[:, k, :])
            # bias tables issued after the V/QK-critical inputs so their big
            # transfers don't hog the DMA engines while V is data-starved
            load_bias(0)
            load_bias(1)

            # ---- building blocks ----
            def qk_block(j, which):
                mmq = ps_mm.tile([P, N], f32, tag="mm",
                                 name=f"mm{'qk'[which]}{j}")
                off = which * C + j * P
                for k in range(CT):
                    for qc in range(QC):
                        nc.tensor.matmul(
                            mmq[:, qc * QN:(qc + 1) * QN],
                            wqk[:, k, off:off + P],
                            xT[:, k, qc * QN:(qc + 1) * QN],
                            start=(k == 0), stop=(k == CT - 1))
                if which == 0:
                    nc.vector.tensor_scalar_add(q_t[:, j, :], mmq,
                                                qkb[:, j:j + 1])
                else:
                    nc.vector.tensor_copy(k_t[:, j, :], mmq)

            def s_half(h, pt, lo, hi):
                p0 = (h % 2) * 64
                for kt in range(lo, hi):
                    pss = ps_s.tile([P, N], f32, tag="pss",
                                    name=f"pss{h}_{kt}")
                    for qc in range(QC):
                        nc.tensor.matmul(
                            pss[:, qc * QN:(qc + 1) * QN],
                            k_t[p0:p0 + 64, h // 2, kt * P:(kt + 1) * P],
                            q_t[p0:p0 + 64, h // 2, qc * QN:(qc + 1) * QN],
                            start=True, stop=True)
                    nc.scalar.activation(pt[:, kt, :], pss, AF.Exp,
                                         bias=0.0, scale=1.0)
                    eng = nc.gpsimd if kt == 0 else nc.vector
                    eng.tensor_mul(pt[:, kt, :], pt[:, kt, :],
                                   biasT[h][:, kt, :])

            def av_half(h, pt, mma, lo, hi):
                for kt in range(lo, hi):
                    for qc in range(QC):
                        nc.tensor.matmul(
                            mma[0:HD + 1, qc * QN:(qc + 1) * QN],
                            v_aug[:, kt, h, :],
                            pt[:, kt, qc * QN:(qc + 1) * QN],
                            start=(kt == 0), stop=(kt == KT - 1))

            def evict_block(h, mma, sums):
                p0 = (h % 2) * 64
                nc.vector.tensor_copy(
                    attn_outT[p0:p0 + HD, h // 2, :], mma[0:HD, :])
                nc.scalar.activation(sums[64:65, h % 2, :], mma[64:65, :],
                                     AF.Identity, bias=0.0, scale=1.0)

            def pairnorm(j, sums, last=False):
                # 1/s via fast-approx reciprocal (no ACT table switches),
                # partition-broadcast through a tiny DRAM bounce on the idle
                # SP queue, applied as one fp16 multiply (Pool normally; DVE
                # for the last pair where the chain is the critical path).
                invg = inv_pool.tile([2, N], f32, tag="inv", name=f"inv{j}")
                invh = inv_pool.tile([2, N], f16, tag="invh", name=f"invh{j}")
                nc.sync.dma_start(invg, sums[64:65, :, :])
                nc.vector.reciprocal_approx_fast(invg, invg)
                nc.vector.tensor_copy(invh, invg)
                w = nc.sync.dma_start(inv_d[j], invh)
                pinv = pinv_pool.tile([P, N], f16, tag="pinv",
                                      name=f"pinv{j}")
                r = nc.sync.dma_start(
                    pinv, bass.AP(tensor=inv_d, offset=j * 2 * N,
                                  ap=[[N, 2], [0, 64], [1, N]]))
                _add_dep_helper(r.ins, w.ins, sync=True,
                                reason="inv scratch RAW")
                eng = nc.vector if last else nc.gpsimd
                eng.tensor_mul(attn_outT[:, j, :], attn_outT[:, j, :],
                               pinv)

            # ---- pipelined main stream ----
            qk_block(0, 0)
            qk_block(0, 1)
            for k in range(CT):
                nc.sync.dma_start(wproj[:, k, :], wproj_src[:, k, :])

            def proj_part(j, mmp, ks, stop_at_end):
                for k in ks:
                    for qc in range(QC):
                        nc.tensor.matmul(
                            mmp[:, qc * QN:(qc + 1) * QN],
                            wproj[:, k, j * P:(j + 1) * P],
                            attn_outT[:, k, qc * QN:(qc + 1) * QN],
                            start=(k == 0), stop=(stop_at_end and k == ks[-1]),
                            skip_group_check=True)

            def proj_finish(j, mmp):
                ybt = yb_pool.tile([P, N], f16, tag="yb", name=f"yb{j}")
                nc.scalar.activation(ybt, mmp, AF.Identity,
                                     bias=pbias[:, j:j + 1], scale=1.0)
                nc.sync.dma_start(
                    yT_d[:].rearrange("(j p) t -> p j t", p=P)[:, j, :], ybt)

            # pairnorm(j) is emitted one cycle late so its serial chain
            # (gather -> recip -> cast -> DRAM bounce -> mult) never sits at
            # the head of the Pool/DVE queues blocking the next head's work.
            # S and AV are emitted in 4-kt half-blocks interleaved with other
            # PE work so the exp (ACT) and bias-mult (DVE/Pool) streams get
            # lead time instead of throttling the PE.
            pending = None
            mmp_pre = {}
            for j in range(NJ):
                a, b = 2 * j, 2 * j + 1
                pt_a = pt_pool.tile([P, KT, N], f16, tag="pt", name=f"pta{a}")
                pt_b = pt_pool.tile([P, KT, N], f16, tag="pt", name=f"ptb{b}")
                if j + 1 < NJ:
                    qk_block(j + 1, 0)
                s_half(a, pt_a, 0, 4)
                if j + 1 < NJ:
                    qk_block(j + 1, 1)
                s_half(a, pt_a, 4, 8)
                if a + 2 < NH:
                    load_bias(a + 2)
                mma_a = ps_mm.tile([P, N], f32, tag="mm", name=f"mma{a}")
                av_half(a, pt_a, mma_a, 0, 4)
                s_half(b, pt_b, 0, 4)
                av_half(a, pt_a, mma_a, 4, 8)
                s_half(b, pt_b, 4, 8)
                if b + 2 < NH:
                    load_bias(b + 2)
                sums = sums_pool.tile([65, 2, N], f32, tag="sums",
                                      name=f"sums{j}")
                evict_block(a, mma_a, sums)
                if pending is not None:
                    pairnorm(*pending)
                if j == NJ - 1:
                    # pre-run proj(0) k0..4 while head 11 finishes
                    mmp_pre[0] = ps_s.tile([P, N], f32, tag="pss",
                                           name="mmp0")
                    proj_part(0, mmp_pre[0], list(range(CT - 1)), False)
                mma_b = ps_mm.tile([P, N], f32, tag="mm", name=f"mma{b}")
                av_half(b, pt_b, mma_b, 0, 8)
                evict_block(b, mma_b, sums)
                pending = (j, sums)
            # pre-run proj(1)/proj(2) k0..4 in the (now free) ps_s banks to
            # cover the last pair's normalization chain, which is emitted
            # in between
            mmp_pre[1] = ps_s.tile([P, N], f32, tag="pss", name="mmp1")
            proj_part(1, mmp_pre[1], list(range(CT - 1)), False)
            pairnorm(*pending, last=True)
            mmp_pre[2] = ps_s.tile([P, N], f32, tag="pss", name="mmp2")
            proj_part(2, mmp_pre[2], list(range(CT - 1)), False)

            # ---- projection tail: y^T = wproj^T @ attn_outT ----
            for j in range(3):
                proj_part(j, mmp_pre[j], [CT - 1], True)
                proj_finish(j, mmp_pre[j])
            for j in range(3, CT):
                pool = ps_mm if j == 3 else ps_s
                mmp = pool.tile([P, N], f32,
                                tag="mm" if j == 3 else "pss",
                                name=f"mmp{j}")
                proj_part(j, mmp, list(range(CT)), True)
                proj_finish(j, mmp)

    nc.compile()
    return nc


def _get_nc():
    if "nc" not in _CACHE:
        _CACHE["nc"] = _build()
    return _CACHE["nc"]


def prepare_inputs(x, qkv_w, q_bias, v_bias, proj_w, proj_b, rel_table,
                   rel_index):
    """Host-side resharding/layout prep. Returns per-core input maps."""
    scale = HD ** -0.5
    x = np.asarray(x, np.float32)
    qkv_w = np.asarray(qkv_w, np.float32)
    q_bias = np.asarray(q_bias, np.float32)
    v_bias = np.asarray(v_bias, np.float32)
    proj_w = np.asarray(proj_w, np.float32)
    proj_b = np.asarray(proj_b, np.float32)
    rel_table = np.asarray(rel_table, np.float32)
    rel_index = np.asarray(rel_index)

    wq = qkv_w[0:C, :] * scale          # [o, c] rows scaled
    wk = qkv_w[C:2 * C, :]
    wv = qkv_w[2 * C:3 * C, :]
    wqk = np.ascontiguousarray(
        np.concatenate([wq, wk], axis=0).T).astype(np.float16)   # [c, 2C]
    wv_t = np.ascontiguousarray(wv.T).astype(np.float16)         # [c, C]
    wproj = np.ascontiguousarray(proj_w.T).astype(np.float16)    # [c, co]
    qkb = np.ascontiguousarray(
        (q_bias * scale).reshape(CT, P).astype(np.float32))
    pb = np.ascontiguousarray(proj_b.reshape(CT, P).astype(np.float32))

    # bias[q, k, h] = rel_table[rel_index[q, k]]; we ship exp(biasT[h, k, q])
    # so the kernel can fold the softmax bias multiplicatively into P^T
    bias = rel_table[rel_index.reshape(-1)].reshape(N, N, NH)
    biasT = np.ascontiguousarray(
        np.exp(bias.transpose(2, 1, 0), dtype=np.float32)).astype(np.float16)

    shared = {
        "wqk": wqk, "wv": wv_t, "wproj": wproj, "qkb": qkb,
        "vb": v_bias.astype(np.float32), "pb": pb, "biasT": biasT,
    }
    in_maps = []
    for b in range(B):
        xt = np.ascontiguousarray(x[b].reshape(N, C).T).astype(np.float16)
        in_maps.append({"xT": xt, **shared})
    return in_maps


def kernel(x, qkv_w, q_bias, v_bias, proj_w, proj_b, rel_table, rel_index,
           _trace=False):
    from concourse.bass_utils import run_bass_kernel_spmd

    nc = _get_nc()
    in_maps = prepare_inputs(x, qkv_w, q_bias, v_bias, proj_w, proj_b,
                             rel_table, rel_index)
    kwargs = {}
    if _trace:
        import concourse.bass_utils as _bu
        _bu.upload_artifacts = lambda tmpdir: tmpdir
        kwargs = {"trace": True}
    res = run_bass_kernel_spmd(nc, in_maps, core_ids=list(range(B)), **kwargs)
    out = np.empty((B, WS, WS, C), np.float32)
    for b in range(B):
        out[b] = res.results[b]["yT"].T.reshape(WS, WS, C).astype(np.float32)
    if _trace:
        _CACHE["last_result"] = res
    return out
